# revision 1
# baseline (speedup 1.0000x reference)
"""Trainium2 Bass kernel for nn_Decoder (dense MLP).

Computes out = relu(V @ W1 + b1) @ W2 + b2 for V [262144, 1024],
W1 [1024, 128], W2 [128, 4].

Strategy
--------
Data-parallel over 8 NeuronCores: V is sharded along rows (32768 rows per
core); the small weights are replicated. Each core's V shard is transposed
on the host to [1024, 32768] so the contraction dim (1024) lands on SBUF
partitions with fully contiguous DMA loads — no on-chip transposes.

Per core, the kernel computes h.T = W1.T @ V.T via PSUM-accumulated
matmuls over 8 K-chunks (lhsT = the natural W1 layout), applies
ReLU(+b1) on the scalar engine reading PSUM, then out.T = W2.T @ h.T on
the tensor engine, adds b2 on the vector engine, and stores out.T
[4, 32768] contiguously. The host transposes the gathered outputs back.

Precision modes (KERNEL_MODE env var):
  f32    — plain fp32 matmuls (4 cycles/row on PE).
  bf16   — single-pass bf16 (half the DMA bytes, ~2e-3 rel err).
  f16    — single-pass fp16 (half the DMA bytes, ~3e-4 rel err).
  bf16x2 — hi/lo bf16 split, 3 matmul passes (~5e-6 rel err).
  f16x2  — hi/lo fp16 split, 3 matmul passes (~1e-6, fp32-grade; default).

Measured on HW (8 cores, full size): f16x2 ≈ 460 us, rel err 1.1e-6.
"""

import os
import sys

import numpy as np

for _p in ("/opt/trn_rl_repo", "/root/.axon_site/_ro/trn_rl_repo"):
    if os.path.isdir(_p) and _p not in sys.path:
        sys.path.insert(0, _p)

import concourse.bass as bass
import concourse.mybir as mybir
import concourse.tile as tile
from concourse import bacc
from concourse.bass_utils import run_bass_kernel_spmd

NCORES = 8
NN = 262144
IN_DIM = 1024
HIDDEN = 128
OUT_DIM = 4
R = NN // NCORES  # rows per core

P = 128           # SBUF partitions
KC = IN_DIM // P  # 8 k-chunks
CHUNK = 512       # rows per PSUM accumulation tile (one PSUM bank)
GROUP = 2048      # rows per DMA group
DATA_BUFS = 2     # prefetch depth for V-group tiles

MODE = os.environ.get("KERNEL_MODE", "f16x2")

_TWO_PASS = {"bf16x2", "f16x2"}
_last_results = None  # exposed for test harness (exec_time_ns etc.)


def _moving_dtype(mode):
    if mode in ("bf16", "bf16x2"):
        return mybir.dt.bfloat16
    if mode in ("f16", "f16x2"):
        return mybir.dt.float16
    return mybir.dt.float32


def build_nc(mode=MODE, rows=R):
    """Build the SPMD Bass program for one core."""
    f32 = mybir.dt.float32
    mdt = _moving_dtype(mode)
    two_pass = mode in _TWO_PASS

    nc = bacc.Bacc("TRN2")

    vth_d = nc.declare_dram_parameter("VTH", [IN_DIM, rows], mdt, isOutput=False)
    if two_pass:
        vtl_d = nc.declare_dram_parameter("VTL", [IN_DIM, rows], mdt, isOutput=False)
    w1h_d = nc.declare_dram_parameter("W1H", [IN_DIM, HIDDEN], mdt, isOutput=False)
    if two_pass:
        w1l_d = nc.declare_dram_parameter("W1L", [IN_DIM, HIDDEN], mdt, isOutput=False)
    b1_d = nc.declare_dram_parameter("B1", [HIDDEN, 1], f32, isOutput=False)
    if two_pass:
        w2h_d = nc.declare_dram_parameter("W2H", [HIDDEN, OUT_DIM], mdt, isOutput=False)
        w2l_d = nc.declare_dram_parameter("W2L", [HIDDEN, OUT_DIM], mdt, isOutput=False)
    else:
        w2_d = nc.declare_dram_parameter("W2", [HIDDEN, OUT_DIM], f32, isOutput=False)
    b2_d = nc.declare_dram_parameter("B2", [OUT_DIM, 1], f32, isOutput=False)
    out_d = nc.declare_dram_parameter("OUT", [OUT_DIM, rows], f32, isOutput=True)

    ngroups = rows // GROUP
    nchunk = GROUP // CHUNK

    with tile.TileContext(nc) as tc:
        with (
            tc.tile_pool(name="const", bufs=1) as cpool,
            tc.tile_pool(name="data", bufs=DATA_BUFS) as dpool,
            tc.tile_pool(name="work", bufs=3) as wpool,
            tc.tile_pool(name="psum1", bufs=4, space="PSUM") as ppool,
            tc.tile_pool(name="psum2", bufs=2, space="PSUM") as opool,
        ):
            # --- constants (loaded once) ---
            w1h_sb = cpool.tile([P, KC, HIDDEN], mdt)
            nc.sync.dma_start(
                w1h_sb[:], w1h_d[:].rearrange("(c p) h -> p c h", p=P)
            )
            if two_pass:
                w1l_sb = cpool.tile([P, KC, HIDDEN], mdt)
                nc.sync.dma_start(
                    w1l_sb[:], w1l_d[:].rearrange("(c p) h -> p c h", p=P)
                )
            b1_sb = cpool.tile([HIDDEN, 1], f32)
            nc.sync.dma_start(b1_sb[:], b1_d[:])
            if two_pass:
                w2h_sb = cpool.tile([HIDDEN, OUT_DIM], mdt)
                nc.sync.dma_start(w2h_sb[:], w2h_d[:])
                w2l_sb = cpool.tile([HIDDEN, OUT_DIM], mdt)
                nc.sync.dma_start(w2l_sb[:], w2l_d[:])
            else:
                w2_sb = cpool.tile([HIDDEN, OUT_DIM], f32)
                nc.sync.dma_start(w2_sb[:], w2_d[:])
            b2_sb = cpool.tile([OUT_DIM, 1], f32)
            nc.sync.dma_start(b2_sb[:], b2_d[:])

            vth_view = vth_d[:].rearrange("(c p) (g n) -> g p c n", p=P, n=GROUP)
            if two_pass:
                vtl_view = vtl_d[:].rearrange("(c p) (g n) -> g p c n", p=P, n=GROUP)
            out_view = out_d[:].rearrange("o (m n) -> m o n", n=CHUNK)

            for g in range(ngroups):
                vth = dpool.tile([P, KC, GROUP], mdt, tag="vth")
                vtl = None
                if two_pass:
                    vtl = dpool.tile([P, KC, GROUP], mdt, tag="vtl")
                if g == 0:
                    # split the first group per-chunk (hi/lo interleaved)
                    # so PE starts as early as possible
                    for u in range(nchunk):
                        slv = slice(u * CHUNK, (u + 1) * CHUNK)
                        nc.sync.dma_start(vth[:, :, slv], vth_view[g][:, :, slv])
                        if two_pass:
                            nc.sync.dma_start(vtl[:, :, slv], vtl_view[g][:, :, slv])
                else:
                    nc.sync.dma_start(vth[:], vth_view[g])
                    if two_pass:
                        nc.sync.dma_start(vtl[:], vtl_view[g])

                for u in range(nchunk):
                    sl = slice(u * CHUNK, (u + 1) * CHUNK)
                    mms = []
                    for c in range(KC):
                        mms.append((w1h_sb[:, c, :], vth[:, c, sl]))
                        if two_pass:
                            mms.append((w1h_sb[:, c, :], vtl[:, c, sl]))
                            mms.append((w1l_sb[:, c, :], vth[:, c, sl]))

                    ps = ppool.tile([HIDDEN, CHUNK], f32, tag="ps")
                    n_mm = len(mms)
                    for i, (wap, vap) in enumerate(mms):
                        nc.tensor.matmul(
                            ps[:], wap, vap,
                            start=(i == 0), stop=(i == n_mm - 1),
                        )

                    po = opool.tile([OUT_DIM, CHUNK], f32, tag="po")
                    if two_pass:
                        # Split h into hi/lo halves so layer 2 also runs at
                        # 1 cycle/row while staying fp32-grade:
                        #   hh = mdt(relu(ps+b1));  hl = mdt(relu_f32 - hh)
                        hh = wpool.tile([HIDDEN, CHUNK], mdt, tag="hh")
                        nc.scalar.activation(
                            hh[:], ps[:],
                            mybir.ActivationFunctionType.Relu,
                            bias=b1_sb[:],
                        )
                        hf = wpool.tile([HIDDEN, CHUNK], f32, tag="hf")
                        nc.vector.tensor_scalar(
                            hf[:], ps[:], b1_sb[:], 0.0,
                            op0=mybir.AluOpType.add,
                            op1=mybir.AluOpType.max,
                        )
                        hl = wpool.tile([HIDDEN, CHUNK], mdt, tag="hl")
                        nc.vector.tensor_sub(hl[:], hf[:], hh[:])
                        nc.tensor.matmul(po[:], w2h_sb[:], hh[:], start=True, stop=False)
                        nc.tensor.matmul(po[:], w2h_sb[:], hl[:], start=False, stop=False)
                        nc.tensor.matmul(po[:], w2l_sb[:], hh[:], start=False, stop=True)
                    else:
                        h_sb = wpool.tile([HIDDEN, CHUNK], f32, tag="h")
                        nc.scalar.activation(
                            h_sb[:], ps[:],
                            mybir.ActivationFunctionType.Relu,
                            bias=b1_sb[:],
                        )
                        w2ap, hap = w2_sb[:], h_sb[:]
                        nc.tensor.matmul(po[:], w2ap, hap, start=True, stop=True)

                    o_sb = wpool.tile([OUT_DIM, CHUNK], f32, tag="o")
                    nc.vector.tensor_scalar_add(o_sb[:], po[:], b2_sb[:])

                    nc.scalar.dma_start(out_view[g * nchunk + u], o_sb[:])

    return nc


def _split_hi_lo(x, np_dt):
    hi = x.astype(np_dt)
    lo = (x - hi.astype(np.float32)).astype(np_dt)
    return hi, lo


def kernel(V, W1, b1, W2, b2):
    global _last_results
    mode = MODE
    mdt = _moving_dtype(mode)
    np_dt = {
        mybir.dt.float32: np.float32,
        mybir.dt.bfloat16: None,  # filled below (ml_dtypes)
        mybir.dt.float16: np.float16,
    }[mdt]
    if np_dt is None:
        import ml_dtypes

        np_dt = ml_dtypes.bfloat16
    two_pass = mode in _TWO_PASS

    V = np.asarray(V, dtype=np.float32)
    W1 = np.asarray(W1, dtype=np.float32)
    b1 = np.asarray(b1, dtype=np.float32)
    W2 = np.asarray(W2, dtype=np.float32)
    b2 = np.asarray(b2, dtype=np.float32)

    common = {
        "B1": np.ascontiguousarray(b1.reshape(HIDDEN, 1)),
        "B2": np.ascontiguousarray(b2.reshape(OUT_DIM, 1)),
    }
    if two_pass:
        common["W1H"], common["W1L"] = _split_hi_lo(W1, np_dt)
        common["W2H"], common["W2L"] = _split_hi_lo(W2, np_dt)
    else:
        common["W1H"] = W1.astype(np_dt)
        common["W2"] = np.ascontiguousarray(W2)

    in_maps = []
    for c in range(NCORES):
        shard = V[c * R : (c + 1) * R]  # [R, IN_DIM]
        if two_pass:
            hi, lo = _split_hi_lo(shard, np_dt)
            m = {
                "VTH": np.ascontiguousarray(hi.T),
                "VTL": np.ascontiguousarray(lo.T),
            }
        else:
            m = {"VTH": np.ascontiguousarray(shard.T.astype(np_dt))}
        m.update(common)
        in_maps.append(m)

    nc = build_nc(mode, R)
    nc.finalize()
    res = run_bass_kernel_spmd(nc, in_maps, list(range(NCORES)))
    _last_results = res

    out = np.concatenate(
        [np.asarray(r["OUT"]).T for r in res.results], axis=0
    ).astype(np.float32)
    return out



# revision 2
# speedup vs baseline: 1.7229x; 1.7229x over previous
"""Trainium2 Bass kernel for nn_Decoder (dense MLP).

Computes out = relu(V @ W1 + b1) @ W2 + b2 for V [262144, 1024],
W1 [1024, 128], W2 [128, 4].

Strategy
--------
Data-parallel over 8 NeuronCores: V is sharded along rows (32768 rows per
core); the small weights are replicated. Each core's V shard is transposed
on the host to [1024, 32768] so the contraction dim (1024) lands on SBUF
partitions with fully contiguous DMA loads — no on-chip transposes.

Per core, the kernel computes h.T = W1.T @ V.T via PSUM-accumulated
matmuls over 8 K-chunks (lhsT = the natural W1 layout), applies
ReLU(+b1) on the scalar engine reading PSUM, then out.T = W2.T @ h.T on
the tensor engine, adds b2 on the vector engine, and stores out.T
[4, 32768] contiguously. The host transposes the gathered outputs back.

Precision modes (KERNEL_MODE env var):
  f32    — plain fp32 matmuls (4 cycles/row on PE).
  bf16   — single-pass bf16 (half the DMA bytes, ~2e-3 rel err).
  f16    — single-pass fp16 (half the DMA bytes, ~3e-4 rel err).
  bf16x2 — hi/lo bf16 split, 3 matmul passes (~5e-6 rel err).
  f16x2  — hi/lo fp16 split, 3 matmul passes (~1e-6, fp32-grade; default).

Measured on HW (8 cores, full size): f16x2 ≈ 460 us, rel err 1.1e-6.
"""

import os
import sys

import numpy as np

for _p in ("/opt/trn_rl_repo", "/root/.axon_site/_ro/trn_rl_repo"):
    if os.path.isdir(_p) and _p not in sys.path:
        sys.path.insert(0, _p)

import concourse.bass as bass
import concourse.mybir as mybir
import concourse.tile as tile
from concourse import bacc
from concourse.bass_utils import run_bass_kernel_spmd

NCORES = 8
NN = 262144
IN_DIM = 1024
HIDDEN = 128
OUT_DIM = 4
R = NN // NCORES  # rows per core

P = 128           # SBUF partitions
KC = IN_DIM // P  # 8 k-chunks
CHUNK = 512       # rows per PSUM accumulation tile (one PSUM bank)
GROUP = 2048      # rows per DMA group
DATA_BUFS = 2     # prefetch depth for V-group tiles

MODE = os.environ.get("KERNEL_MODE", "f16")

_TWO_PASS = {"bf16x2", "f16x2"}
_last_results = None  # exposed for test harness (exec_time_ns etc.)


def _moving_dtype(mode):
    if mode in ("bf16", "bf16x2"):
        return mybir.dt.bfloat16
    if mode in ("f16", "f16x2"):
        return mybir.dt.float16
    return mybir.dt.float32


def build_nc(mode=MODE, rows=R):
    """Build the SPMD Bass program for one core."""
    f32 = mybir.dt.float32
    mdt = _moving_dtype(mode)
    two_pass = mode in _TWO_PASS

    nc = bacc.Bacc("TRN2")

    vth_d = nc.declare_dram_parameter("VTH", [IN_DIM, rows], mdt, isOutput=False)
    if two_pass:
        vtl_d = nc.declare_dram_parameter("VTL", [IN_DIM, rows], mdt, isOutput=False)
    w1h_d = nc.declare_dram_parameter("W1H", [IN_DIM, HIDDEN], mdt, isOutput=False)
    if two_pass:
        w1l_d = nc.declare_dram_parameter("W1L", [IN_DIM, HIDDEN], mdt, isOutput=False)
    b1_d = nc.declare_dram_parameter("B1", [HIDDEN, 1], f32, isOutput=False)
    if two_pass:
        w2h_d = nc.declare_dram_parameter("W2H", [HIDDEN, OUT_DIM], mdt, isOutput=False)
        w2l_d = nc.declare_dram_parameter("W2L", [HIDDEN, OUT_DIM], mdt, isOutput=False)
    else:
        w2_d = nc.declare_dram_parameter("W2", [HIDDEN, OUT_DIM], f32, isOutput=False)
    b2_d = nc.declare_dram_parameter("B2", [OUT_DIM, 1], f32, isOutput=False)
    out_d = nc.declare_dram_parameter("OUT", [OUT_DIM, rows], f32, isOutput=True)

    ngroups = rows // GROUP
    nchunk = GROUP // CHUNK

    with tile.TileContext(nc) as tc:
        with (
            tc.tile_pool(name="const", bufs=1) as cpool,
            tc.tile_pool(name="data", bufs=DATA_BUFS) as dpool,
            tc.tile_pool(name="work", bufs=3) as wpool,
            tc.tile_pool(name="psum1", bufs=4, space="PSUM") as ppool,
            tc.tile_pool(name="psum2", bufs=2, space="PSUM") as opool,
        ):
            # --- constants (loaded once) ---
            w1h_sb = cpool.tile([P, KC, HIDDEN], mdt)
            nc.sync.dma_start(
                w1h_sb[:], w1h_d[:].rearrange("(c p) h -> p c h", p=P)
            )
            if two_pass:
                w1l_sb = cpool.tile([P, KC, HIDDEN], mdt)
                nc.sync.dma_start(
                    w1l_sb[:], w1l_d[:].rearrange("(c p) h -> p c h", p=P)
                )
            b1_sb = cpool.tile([HIDDEN, 1], f32)
            nc.sync.dma_start(b1_sb[:], b1_d[:])
            if two_pass:
                w2h_sb = cpool.tile([HIDDEN, OUT_DIM], mdt)
                nc.sync.dma_start(w2h_sb[:], w2h_d[:])
                w2l_sb = cpool.tile([HIDDEN, OUT_DIM], mdt)
                nc.sync.dma_start(w2l_sb[:], w2l_d[:])
            else:
                w2_sb = cpool.tile([HIDDEN, OUT_DIM], f32)
                nc.sync.dma_start(w2_sb[:], w2_d[:])
            b2_sb = cpool.tile([OUT_DIM, 1], f32)
            nc.sync.dma_start(b2_sb[:], b2_d[:])

            vth_view = vth_d[:].rearrange("(c p) (g n) -> g p c n", p=P, n=GROUP)
            if two_pass:
                vtl_view = vtl_d[:].rearrange("(c p) (g n) -> g p c n", p=P, n=GROUP)
            out_view = out_d[:].rearrange("o (m n) -> m o n", n=CHUNK)

            for g in range(ngroups):
                vth = dpool.tile([P, KC, GROUP], mdt, tag="vth")
                vtl = None
                if two_pass:
                    vtl = dpool.tile([P, KC, GROUP], mdt, tag="vtl")
                if g == 0:
                    # split the first group per-chunk (hi/lo interleaved)
                    # so PE starts as early as possible
                    for u in range(nchunk):
                        slv = slice(u * CHUNK, (u + 1) * CHUNK)
                        nc.sync.dma_start(vth[:, :, slv], vth_view[g][:, :, slv])
                        if two_pass:
                            nc.sync.dma_start(vtl[:, :, slv], vtl_view[g][:, :, slv])
                else:
                    nc.sync.dma_start(vth[:], vth_view[g])
                    if two_pass:
                        nc.sync.dma_start(vtl[:], vtl_view[g])

                for u in range(nchunk):
                    sl = slice(u * CHUNK, (u + 1) * CHUNK)
                    mms = []
                    for c in range(KC):
                        mms.append((w1h_sb[:, c, :], vth[:, c, sl]))
                        if two_pass:
                            mms.append((w1h_sb[:, c, :], vtl[:, c, sl]))
                            mms.append((w1l_sb[:, c, :], vth[:, c, sl]))

                    ps = ppool.tile([HIDDEN, CHUNK], f32, tag="ps")
                    n_mm = len(mms)
                    for i, (wap, vap) in enumerate(mms):
                        nc.tensor.matmul(
                            ps[:], wap, vap,
                            start=(i == 0), stop=(i == n_mm - 1),
                        )

                    po = opool.tile([OUT_DIM, CHUNK], f32, tag="po")
                    if two_pass:
                        # Split h into hi/lo halves so layer 2 also runs at
                        # 1 cycle/row while staying fp32-grade:
                        #   hh = mdt(relu(ps+b1));  hl = mdt(relu_f32 - hh)
                        hh = wpool.tile([HIDDEN, CHUNK], mdt, tag="hh")
                        nc.scalar.activation(
                            hh[:], ps[:],
                            mybir.ActivationFunctionType.Relu,
                            bias=b1_sb[:],
                        )
                        hf = wpool.tile([HIDDEN, CHUNK], f32, tag="hf")
                        nc.vector.tensor_scalar(
                            hf[:], ps[:], b1_sb[:], 0.0,
                            op0=mybir.AluOpType.add,
                            op1=mybir.AluOpType.max,
                        )
                        hl = wpool.tile([HIDDEN, CHUNK], mdt, tag="hl")
                        nc.vector.tensor_sub(hl[:], hf[:], hh[:])
                        nc.tensor.matmul(po[:], w2h_sb[:], hh[:], start=True, stop=False)
                        nc.tensor.matmul(po[:], w2h_sb[:], hl[:], start=False, stop=False)
                        nc.tensor.matmul(po[:], w2l_sb[:], hh[:], start=False, stop=True)
                    else:
                        h_sb = wpool.tile([HIDDEN, CHUNK], f32, tag="h")
                        nc.scalar.activation(
                            h_sb[:], ps[:],
                            mybir.ActivationFunctionType.Relu,
                            bias=b1_sb[:],
                        )
                        w2ap, hap = w2_sb[:], h_sb[:]
                        nc.tensor.matmul(po[:], w2ap, hap, start=True, stop=True)

                    o_sb = wpool.tile([OUT_DIM, CHUNK], f32, tag="o")
                    nc.vector.tensor_scalar_add(o_sb[:], po[:], b2_sb[:])

                    nc.scalar.dma_start(out_view[g * nchunk + u], o_sb[:])

    return nc


def _split_hi_lo(x, np_dt):
    hi = x.astype(np_dt)
    lo = (x - hi.astype(np.float32)).astype(np_dt)
    return hi, lo


def kernel(V, W1, b1, W2, b2):
    global _last_results
    mode = MODE
    mdt = _moving_dtype(mode)
    np_dt = {
        mybir.dt.float32: np.float32,
        mybir.dt.bfloat16: None,  # filled below (ml_dtypes)
        mybir.dt.float16: np.float16,
    }[mdt]
    if np_dt is None:
        import ml_dtypes

        np_dt = ml_dtypes.bfloat16
    two_pass = mode in _TWO_PASS

    V = np.asarray(V, dtype=np.float32)
    W1 = np.asarray(W1, dtype=np.float32)
    b1 = np.asarray(b1, dtype=np.float32)
    W2 = np.asarray(W2, dtype=np.float32)
    b2 = np.asarray(b2, dtype=np.float32)

    common = {
        "B1": np.ascontiguousarray(b1.reshape(HIDDEN, 1)),
        "B2": np.ascontiguousarray(b2.reshape(OUT_DIM, 1)),
    }
    if two_pass:
        common["W1H"], common["W1L"] = _split_hi_lo(W1, np_dt)
        common["W2H"], common["W2L"] = _split_hi_lo(W2, np_dt)
    else:
        common["W1H"] = W1.astype(np_dt)
        common["W2"] = np.ascontiguousarray(W2)

    in_maps = []
    for c in range(NCORES):
        shard = V[c * R : (c + 1) * R]  # [R, IN_DIM]
        if two_pass:
            hi, lo = _split_hi_lo(shard, np_dt)
            m = {
                "VTH": np.ascontiguousarray(hi.T),
                "VTL": np.ascontiguousarray(lo.T),
            }
        else:
            m = {"VTH": np.ascontiguousarray(shard.T.astype(np_dt))}
        m.update(common)
        in_maps.append(m)

    nc = build_nc(mode, R)
    nc.finalize()
    res = run_bass_kernel_spmd(nc, in_maps, list(range(NCORES)))
    _last_results = res

    out = np.concatenate(
        [np.asarray(r["OUT"]).T for r in res.results], axis=0
    ).astype(np.float32)
    return out



# revision 6
# speedup vs baseline: 2.8464x; 1.6521x over previous
"""Trainium2 Bass kernel for nn_Decoder (dense MLP).

Computes out = relu(V @ W1 + b1) @ W2 + b2 for V [262144, 1024],
W1 [1024, 128], W2 [128, 4].

Strategy
--------
Data-parallel over 8 NeuronCores: V is sharded along rows (32768 rows per
core); the small weights are replicated. Each core's V shard is transposed
and blocked on the host to [128, ngroups, 8, 2048] so the contraction dim
lands on SBUF partitions with one fully-contiguous 16KB-per-partition DMA
descriptor per group — no on-chip transposes.

Per core the kernel computes h.T = W1.T @ V.T with a k-outer loop: for
each 2048-row group, each of the 8 stationary W1 k-chunks is loaded once
and streamed against all four 512-row column chunks (PSUM-accumulated
across k). ReLU(+b1) runs on the scalar engine (PSUM -> fp16 SBUF), then
out.T = W2.T @ h.T on the tensor engine (fp16), +b2 on the vector engine
into a [4, 2048] group buffer stored contiguously by the Pool engine.
The host transposes the gathered [4, 32768] outputs back.

Precision modes (KERNEL_MODE env var):
  f8  — V cast to fp8 e3m4 (1 byte/elem DMA), weights fp16.  Rel err
        ~1.4e-2 (tolerance is 2e-2): V's 4-bit mantissa dominates; the
        exact sim of this quantization on the real inputs measures 0.0140.
  f16 — V cast to fp16 (2 bytes/elem DMA), weights fp16. Rel err ~3e-4.
"""

import os
import sys

import numpy as np

for _p in ("/opt/trn_rl_repo", "/root/.axon_site/_ro/trn_rl_repo"):
    if os.path.isdir(_p) and _p not in sys.path:
        sys.path.insert(0, _p)

import concourse.bass as bass
import concourse.mybir as mybir
import concourse.tile as tile
from concourse import bacc
from concourse.bass_utils import run_bass_kernel_spmd

NCORES = 8
NN = 262144
IN_DIM = 1024
HIDDEN = 128
OUT_DIM = 4
R = NN // NCORES  # rows per core

P = 128           # SBUF partitions
KC = IN_DIM // P  # 8 k-chunks
CHUNK = 512       # rows per PSUM accumulation tile (one PSUM bank)
GROUP = 2048      # rows per DMA group / k-outer supergroup
NG = R // GROUP   # 16 groups
NU = GROUP // CHUNK  # 4 chunks per group
DATA_BUFS = 4     # prefetch depth for V-group tiles

MODE = os.environ.get("KERNEL_MODE", "f8")

_last_results = None  # exposed for test harness (exec_time_ns etc.)


def _v_dtype(mode):
    return mybir.dt.float8e3 if mode == "f8" else mybir.dt.float16


def build_nc(mode=MODE, rows=R):
    """Build the SPMD Bass program for one core."""
    f32 = mybir.dt.float32
    f16 = mybir.dt.float16
    vdt = _v_dtype(mode)

    nc = bacc.Bacc("TRN2")

    vt_d = nc.declare_dram_parameter("VT", [P, NG * KC * GROUP], vdt, isOutput=False)
    w1_d = nc.declare_dram_parameter("W1", [IN_DIM, HIDDEN], f16, isOutput=False)
    b1_d = nc.declare_dram_parameter("B1", [HIDDEN, 1], f32, isOutput=False)
    w2_d = nc.declare_dram_parameter("W2", [HIDDEN, OUT_DIM], f16, isOutput=False)
    b2_d = nc.declare_dram_parameter("B2", [OUT_DIM, 1], f32, isOutput=False)
    out_d = nc.declare_dram_parameter("OUT", [OUT_DIM, rows], f32, isOutput=True)

    with tile.TileContext(nc) as tc:
        with (
            tc.tile_pool(name="const", bufs=1) as cpool,
            tc.tile_pool(name="data", bufs=DATA_BUFS) as dpool,
            tc.tile_pool(name="hbuf", bufs=2 * NU) as hpool,
            tc.tile_pool(name="obuf", bufs=2) as obpool,
            tc.tile_pool(name="psum1", bufs=6, space="PSUM") as ppool,
            tc.tile_pool(name="psum2", bufs=2, space="PSUM") as opool,
        ):
            # --- constants (loaded once) ---
            w1_sb = cpool.tile([P, KC, HIDDEN], f16)
            nc.sync.dma_start(w1_sb[:], w1_d[:].rearrange("(c p) h -> p c h", p=P))
            b1_sb = cpool.tile([HIDDEN, 1], f32)
            nc.sync.dma_start(b1_sb[:], b1_d[:])
            w2_sb = cpool.tile([HIDDEN, OUT_DIM], f16)
            nc.sync.dma_start(w2_sb[:], w2_d[:])
            b2_sb = cpool.tile([OUT_DIM, 1], f32)
            nc.sync.dma_start(b2_sb[:], b2_d[:])

            vt_view = vt_d[:].rearrange("p (g c n) -> g p c n", g=NG, c=KC, n=GROUP)
            out_view = out_d[:].rearrange("o (g n) -> g o n", n=GROUP)

            # mm2 work for the previous group, delayed so the PE never
            # waits on the scalar-engine ReLU evacuation:
            #   pending = (hh tiles, o_sb buffer, group index)
            pending = None

            def emit_mm2_step(pend, u):
                hh_tiles, o_sb, _g = pend
                po = opool.tile([OUT_DIM, CHUNK], f32, tag="po")
                nc.tensor.matmul(po[:], w2_sb[:], hh_tiles[u][:], start=True, stop=True)
                nc.vector.tensor_scalar_add(
                    o_sb[:, u * CHUNK : (u + 1) * CHUNK], po[:], b2_sb[:]
                )

            def flush_mm2(pend):
                hh_tiles, o_sb, g = pend
                nc.gpsimd.dma_start(out_view[g], o_sb[:])

            for g in range(NG):
                vt = dpool.tile([P, KC, GROUP], vdt, tag="vt")
                if g == 0:
                    # split the first group per k-chunk so the PE starts
                    # after ~1/8 of the group load
                    for c in range(KC):
                        nc.sync.dma_start(vt[:, c, :], vt_view[g][:, c, :])
                else:
                    nc.sync.dma_start(vt[:], vt_view[g])

                ps_tiles = [
                    ppool.tile([HIDDEN, CHUNK], f32, tag="ps", name=f"ps{u}")
                    for u in range(NU)
                ]
                for c in range(KC):
                    w_ap = w1_sb[:, c, :]
                    for u in range(NU):
                        nc.tensor.matmul(
                            ps_tiles[u][:],
                            w_ap,
                            vt[:, c, u * CHUNK : (u + 1) * CHUNK],
                            start=(c == 0),
                            stop=(c == KC - 1),
                        )
                    # interleave the previous group's tiny layer-2 matmuls
                    # between k-chunks (one per chunk, c=1..NU)
                    if pending is not None and 1 <= c <= NU:
                        emit_mm2_step(pending, c - 1)
                if pending is not None:
                    flush_mm2(pending)

                hh_tiles = []
                for u in range(NU):
                    hh = hpool.tile([HIDDEN, CHUNK], f16, tag="hh")
                    nc.scalar.activation(
                        hh[:], ps_tiles[u][:],
                        mybir.ActivationFunctionType.Relu,
                        bias=b1_sb[:],
                    )
                    hh_tiles.append(hh)
                o_sb = obpool.tile([OUT_DIM, GROUP], f32, tag="o")
                pending = (hh_tiles, o_sb, g)

            for u in range(NU):
                emit_mm2_step(pending, u)
            flush_mm2(pending)

    return nc


def kernel(V, W1, b1, W2, b2):
    global _last_results
    mode = MODE
    if mode == "f8":
        import ml_dtypes

        np_vdt = ml_dtypes.float8_e3m4
    else:
        np_vdt = np.float16

    V = np.asarray(V, dtype=np.float32)
    W1 = np.asarray(W1, dtype=np.float32)
    b1 = np.asarray(b1, dtype=np.float32)
    W2 = np.asarray(W2, dtype=np.float32)
    b2 = np.asarray(b2, dtype=np.float32)

    common = {
        "W1": W1.astype(np.float16),
        "B1": np.ascontiguousarray(b1.reshape(HIDDEN, 1)),
        "W2": W2.astype(np.float16),
        "B2": np.ascontiguousarray(b2.reshape(OUT_DIM, 1)),
    }

    in_maps = []
    for c in range(NCORES):
        shard = V[c * R : (c + 1) * R]  # [R, IN_DIM]
        # [IN_DIM, R] -> (c, p, g, n) -> [P, NG, KC, GROUP], one contiguous
        # (KC*GROUP)-run per (partition, group)
        vt = shard.T.reshape(KC, P, NG, GROUP).transpose(1, 2, 0, 3)
        m = {"VT": vt.astype(np_vdt).reshape(P, NG * KC * GROUP)}
        m.update(common)
        in_maps.append(m)

    nc = build_nc(mode, R)
    nc.finalize()
    res = run_bass_kernel_spmd(nc, in_maps, list(range(NCORES)))
    _last_results = res

    out = np.concatenate(
        [np.asarray(r["OUT"]).T for r in res.results], axis=0
    ).astype(np.float32)
    return out
